# revision 9
# baseline (speedup 1.0000x reference)
"""AttentionBasedDGI forward loss on 8 Trainium2 NeuronCores.

Math (reference):
  M = D^-1/2 (A+I) D^-1/2   (dense normalized multigraph adjacency, built on host)
  z1 = relu(M @ x @ W1 + b1)            [2, N, F]
  z  = M @ z1 @ W2 + b2                 [2, N, F]
  multi-head self-attention over pos_z=z[0], global mean -> scalar g
  loss = mean softplus(-g*rowsum(pos_z)) + mean softplus(g*rowsum(neg_z))

Distribution: rows (destination nodes) sharded 512/core.  Layer outputs are
computed feature-major ("T" layout: [F, rows]) so every matmul's stationary
operand is in natural layout; z1 is transposed back to node-major with PE
transposes and all-gathered; pos_z^T is all-gathered for K/V.  Attention is
computed in S^T = [keys, queries] layout with the softmax denominator folded
into the O^T matmul as an extra ones-column of V.  The scalar epilogue (g,
logits, softplus) runs on device; the host only sums 8 per-core partials.
"""

import sys

if '/opt/trn_rl_repo' not in sys.path:
    sys.path.insert(0, '/opt/trn_rl_repo')

import numpy as np
import ml_dtypes

import concourse.bass as bass
import concourse.mybir as mybir
import concourse.tile as tile
from concourse.bass_utils import run_bass_kernel_spmd

BF16 = mybir.dt.bfloat16
F32 = mybir.dt.float32
AF = mybir.ActivationFunctionType

NC_ = 8          # cores
N = 4096         # nodes
S = 512          # rows per core
F = 256          # features
NH = 4           # heads
D = 64           # head dim
KT = 32          # 128-row source tiles


class SplitDrainTileContext(tile.TileContext):
    """Tail drain split into one drain per proc — walrus rejects a CTRL
    instruction carrying more than one sync-wait command."""

    def _drain_and_barrier(self, tick_clock, wait_clock):
        from concourse.vector_clock import VectorClock, ScopedClock
        from concourse.tile_scheduler import N_PROCS

        full = tick_clock.global_clock
        for p in range(N_PROCS):
            if full[p] > 0:
                vec = [full[q] if q == p else 0 for q in range(N_PROCS)]
                drain_inst = self.nc.sync.drain()
                wait_clock.add_sem_waits(
                    drain_inst.ins, ScopedClock({None: VectorClock(vec)})
                )
        self.nc.all_engine_barrier()
        assert self.sems is not None
        popped = self.nc._tile_sem_poison_stack.pop()
        assert popped is self._sem_poison
        self.nc.clear_and_free_semaphores(list(self.sems.allocated().values()))
        self.nc.all_engine_barrier()


_COMPUTE_INSTS = None


def _split_excess_waits(nc, max_compute=1, max_other=1):
    """walrus's codegen rejects instructions carrying more than a couple of
    embedded sync-wait commands (1 for CTRL-type, ~2 for engine structs).
    Spill excess waits onto same-engine NoOps placed just before the
    instruction — the engine stream serializes them, which is equivalent."""
    ctr = [0]
    for fn in nc.m.functions:
        for bb in fn.blocks:
            insts = bb.instructions
            out = []
            changed = False
            for inst in insts:
                si = inst.sync_info
                waits = list(si.on_wait) if si is not None and si.on_wait else []
                tname = type(inst).__name__
                cap = max_compute if tname in (
                    "InstActivation", "InstTensorTensor", "InstTensorCopy",
                    "InstTensorScalarPtr", "InstTensorReduce", "InstMatmult",
                    "InstLdweights", "InstReciprocal", "InstCopy",
                ) else max_other
                if len(waits) > cap:
                    excess, keep = waits[:-cap], waits[-cap:]
                    for w in excess:
                        nop = mybir.InstNoOp(name=f"WSPLIT-{ctr[0]}", ins=[], outs=[])
                        ctr[0] += 1
                        nop.engine = inst.engine
                        nop.bass_nofuse = True
                        nop.sync_info = mybir.SyncInfo(on_wait=[w], on_update=[])
                        out.append(nop)
                    inst.sync_info = mybir.SyncInfo(
                        on_wait=keep, on_update=list(si.on_update or [])
                    )
                    changed = True
                out.append(inst)
            if changed:
                bb.instructions = out


def build_nc():
    nc = bass.Bass()
    RG = [list(range(NC_))]

    # ---- parameters (per-core).  Layout comments: p = SBUF partition dim ----
    mt_p = nc.declare_dram_parameter("mt", [KT, 128, S], BF16, isOutput=False)
    x_p = nc.declare_dram_parameter("xr", [2, KT, 128, F], BF16, isOutput=False)
    w1_p = nc.declare_dram_parameter("w1", [2, 128, F], BF16, isOutput=False)   # [fi,p,fo]
    w2_p = nc.declare_dram_parameter("w2", [2, 128, F], BF16, isOutput=False)
    b1_p = nc.declare_dram_parameter("b1", [2, 128, 1], F32, isOutput=False)    # [fo,p,1]
    b2_p = nc.declare_dram_parameter("b2", [2, 128, 1], F32, isOutput=False)
    wqt_p = nc.declare_dram_parameter("wqt", [NH, 2, 128, D], BF16, isOutput=False)  # [h,fi,p,d]
    wkt_p = nc.declare_dram_parameter("wkt", [NH, 2, 128, D], BF16, isOutput=False)
    wvt_p = nc.declare_dram_parameter("wvt", [NH, 2, 128, D], BF16, isOutput=False)
    bq_p = nc.declare_dram_parameter("bq", [NH, D, 1], F32, isOutput=False)
    bk_p = nc.declare_dram_parameter("bk", [NH, D, 1], F32, isOutput=False)
    # per-head [bv_h..., 1.0] row for the V-bias/ones matmul: [h, 1, D+1]
    bv_p = nc.declare_dram_parameter("bv", [NH, 1, D + 1], BF16, isOutput=False)
    owt_p = nc.declare_dram_parameter("owt", [2, 128, F], BF16, isOutput=False)  # [hi,p,fo]
    ob_p = nc.declare_dram_parameter("ob", [2, 128, 1], F32, isOutput=False)
    swt_p = nc.declare_dram_parameter("swt", [2, 128, 1], F32, isOutput=False)   # sum_w.T
    sb_p = nc.declare_dram_parameter("sbias", [1, 1], F32, isOutput=False)
    ident_p = nc.declare_dram_parameter("ident", [128, 128], BF16, isOutput=False)
    onesc_p = nc.declare_dram_parameter("onesc", [128, 1], BF16, isOutput=False)
    onesr_p = nc.declare_dram_parameter("onesr", [1, 128], BF16, isOutput=False)
    onesr32_p = nc.declare_dram_parameter("onesr32", [1, 64], F32, isOutput=False)

    part_p = nc.declare_dram_parameter("partial", [1, 1], F32, isOutput=True)

    with SplitDrainTileContext(nc, num_cores=NC_) as tc:
        with (
            tc.tile_pool(name="const", bufs=1) as cpool,
            tc.tile_pool(name="mtx", bufs=1) as mtx,
            tc.tile_pool(name="acts", bufs=1) as acts,
            tc.tile_pool(name="work", bufs=3) as work,
            tc.tile_pool(name="psA", bufs=3, space="PSUM") as psA,
            tc.tile_pool(name="psB", bufs=2, space="PSUM") as psB,
            tc.tile_pool(name="psS", bufs=2, space="PSUM") as psS,
            tc.tile_pool(name="psOne", bufs=1, space="PSUM") as psOne,
            tc.tile_pool(name="dram", bufs=1, space="DRAM") as dram,
        ):
            # ---- constants to SBUF (partition-major layouts) ----
            w1_sb = cpool.tile([128, 2, F], BF16, name="w1_sb")
            w2_sb = cpool.tile([128, 2, F], BF16, name="w2_sb")
            b1_sb = cpool.tile([128, 2, 1], F32, name="b1_sb")
            b2_sb = cpool.tile([128, 2, 1], F32, name="b2_sb")
            wqt_sb = cpool.tile([128, NH, 2, D], BF16, name="wqt_sb")
            wkt_sb = cpool.tile([128, NH, 2, D], BF16, name="wkt_sb")
            wvt_sb = cpool.tile([128, NH, 2, D], BF16, name="wvt_sb")
            bq_sb = cpool.tile([D, NH, 1], F32, name="bq_sb")
            bk_sb = cpool.tile([D, NH, 1], F32, name="bk_sb")
            bv_sb = cpool.tile([1, NH, D + 1], BF16, name="bv_sb")
            owt_sb = cpool.tile([128, 2, F], BF16, name="owt_sb")
            ob_sb = cpool.tile([128, 2, 1], F32, name="ob_sb")
            swt_sb = cpool.tile([128, 2, 1], F32, name="swt_sb")
            sb_sb = cpool.tile([1, 1], F32, name="sb_sb")
            ident_sb = cpool.tile([128, 128], BF16, name="ident_sb")
            onesc_sb = cpool.tile([128, 1], BF16, name="onesc_sb")
            onesr_sb = cpool.tile([1, 128], BF16, name="onesr_sb")
            onesr32_sb = cpool.tile([1, 64], F32, name="onesr32_sb")

            for idx2, (dst, src) in enumerate((
                (w1_sb, w1_p), (w2_sb, w2_p), (b1_sb, b1_p), (b2_sb, b2_p),
                (owt_sb, owt_p), (ob_sb, ob_p), (swt_sb, swt_p),
            )):
                for i in range(2):
                    nc.sync.dma_start(dst[:, i], src[i])
            for dst, src in ((wqt_sb, wqt_p), (wkt_sb, wkt_p), (wvt_sb, wvt_p)):
                for h in range(NH):
                    for fi in range(2):
                        nc.sync.dma_start(dst[:, h, fi], src[h, fi])
            for dst, src in ((bq_sb, bq_p), (bk_sb, bk_p), (bv_sb, bv_p)):
                for h in range(NH):
                    nc.sync.dma_start(dst[:, h], src[h])
            for dst, src in (
                (sb_sb, sb_p), (ident_sb, ident_p), (onesc_sb, onesc_p),
                (onesr_sb, onesr_p), (onesr32_sb, onesr32_p),
            ):
                nc.sync.dma_start(dst[...], src[...])

            # ---- load Mt and x ----
            mt_sb = mtx.tile([128, KT, S], BF16, name="mt_sb")
            x_sb = mtx.tile([128, 2, KT, F], BF16, name="x_sb")
            for kt in range(KT):
                nc.sync.dma_start(mt_sb[:, kt, :], mt_p[kt])
            for v in range(2):
                for kt in range(KT):
                    nc.sync.dma_start(x_sb[:, v, kt, :], x_p[v, kt])

            # ---- layer 1: P1T = (M_shard @ x)^T; z1T = relu(...) ----
            z1t_sb = acts.tile([128, 2, 2, S], BF16, name="z1t_sb")  # [p, v, fo, m]
            for v in range(2):
                p1t_sb = acts.tile([128, 2, S], BF16, name=f"p1t_sb{v}", tag=f"p1t{v}")
                for ft in range(2):
                    ps = psA.tile([128, S], F32, name="ps_p1t", tag="big")
                    for kt in range(KT):
                        nc.tensor.matmul(
                            ps[...],
                            x_sb[:, v, kt, ft * 128:(ft + 1) * 128],
                            mt_sb[:, kt, :],
                            start=(kt == 0), stop=(kt == KT - 1),
                        )
                    nc.vector.tensor_copy(p1t_sb[:, ft], ps[...])
                for fo in range(2):
                    ps = psA.tile([128, S], F32, name="ps_z1t", tag="big")
                    for fi in range(2):
                        nc.tensor.matmul(
                            ps[...],
                            w1_sb[:, fi, fo * 128:(fo + 1) * 128],
                            p1t_sb[:, fi],
                            start=(fi == 0), stop=(fi == 1),
                        )
                    nc.scalar.activation(z1t_sb[:, v, fo], ps[...], AF.Relu, bias=b1_sb[:, fo])

            # ---- transpose z1T -> node-major, allgather ----
            ag1_in = dram.tile([2 * S, F], BF16, name="ag1_in")
            ag1_out = dram.tile([NC_ * 2 * S, F], BF16, name="ag1_out", addr_space="Shared")
            for v in range(2):
                for rt in range(4):
                    z1n_sb = work.tile([128, F], BF16, name="z1n_sb", tag="z1n", bufs=2)
                    for fo in range(2):
                        pst = psB.tile([128, 128], BF16, name="ps_tr", tag="sm")
                        nc.tensor.transpose(
                            pst[...],
                            z1t_sb[:, v, fo, rt * 128:(rt + 1) * 128],
                            ident_sb[...],
                        )
                        nc.vector.tensor_copy(z1n_sb[:, fo * 128:(fo + 1) * 128], pst[...])
                    nc.sync.dma_start(
                        ag1_in[v * S + rt * 128: v * S + (rt + 1) * 128, :], z1n_sb[...]
                    )
            nc.gpsimd.collective_compute(
                "AllGather", mybir.AluOpType.bypass,
                replica_groups=RG, ins=[ag1_in.opt()], outs=[ag1_out.opt()],
            )

            # ---- load z1 full (node-major) ----
            z1f_sb = mtx.tile([128, 2, KT, F], BF16, name="z1f_sb")
            ag1v = ag1_out.rearrange("(c v r) f -> c v r f", c=NC_, v=2)
            for v in range(2):
                for kt in range(KT):
                    kc, lr = kt // 4, kt % 4
                    nc.sync.dma_start(
                        z1f_sb[:, v, kt, :], ag1v[kc, v, lr * 128:(lr + 1) * 128, :]
                    )

            # ---- layer 2 -> z2T (pos/neg, feature-major) ----
            z2t_sb = acts.tile([128, 2, 2, S], BF16, name="z2t_sb")  # [p, v, fo, m]
            for v in range(2):
                p2t_sb = acts.tile([128, 2, S], BF16, name=f"p2t_sb{v}", tag=f"p2t{v}")
                for ft in range(2):
                    ps = psA.tile([128, S], F32, name="ps_p2t", tag="big")
                    for kt in range(KT):
                        nc.tensor.matmul(
                            ps[...],
                            z1f_sb[:, v, kt, ft * 128:(ft + 1) * 128],
                            mt_sb[:, kt, :],
                            start=(kt == 0), stop=(kt == KT - 1),
                        )
                    nc.vector.tensor_copy(p2t_sb[:, ft], ps[...])
                for fo in range(2):
                    ps = psA.tile([128, S], F32, name="ps_z2t", tag="big")
                    for fi in range(2):
                        nc.tensor.matmul(
                            ps[...],
                            w2_sb[:, fi, fo * 128:(fo + 1) * 128],
                            p2t_sb[:, fi],
                            start=(fi == 0), stop=(fi == 1),
                        )
                    nc.scalar.activation(z2t_sb[:, v, fo], ps[...], AF.Identity, bias=b2_sb[:, fo])

            # ---- row sums of pos_z / neg_z ----
            srow_sb = work.tile([1, 2, S], F32, name="srow_sb", tag="srow", bufs=1)
            for v in range(2):
                ps1 = psOne.tile([1, S], F32, name="ps_srow", tag="one")
                for fi in range(2):
                    nc.tensor.matmul(
                        ps1[...], onesc_sb[...], z2t_sb[:, v, fi],
                        start=(fi == 0), stop=(fi == 1),
                    )
                nc.vector.tensor_copy(srow_sb[:, v], ps1[...])

            # ---- allgather pos_z^T ----
            ag2_in = dram.tile([F, S], BF16, name="ag2_in")
            ag2_out = dram.tile([NC_ * F, S], BF16, name="ag2_out", addr_space="Shared")
            for fo in range(2):
                nc.sync.dma_start(ag2_in[fo * 128:(fo + 1) * 128, :], z2t_sb[:, 0, fo])
            nc.gpsimd.collective_compute(
                "AllGather", mybir.AluOpType.bypass,
                replica_groups=RG, ins=[ag2_in.opt()], outs=[ag2_out.opt()],
            )
            pztf_sb = mtx.tile([128, 2, N], BF16, name="pztf_sb")
            ag2v = ag2_out.rearrange("(c ft p) m -> c ft p m", c=NC_, ft=2)
            for fo in range(2):
                for c in range(NC_):
                    nc.sync.dma_start(pztf_sb[:, fo, c * S:(c + 1) * S], ag2v[c, fo])

            # ---- K^T (all nodes) and q^T (own rows) per head ----
            kt_sb = acts.tile([D, NH, N], BF16, name="kt_sb")
            qt_sb = acts.tile([D, NH, S], BF16, name="qt_sb")
            for h in range(NH):
                for nb in range(8):
                    psk = psB.tile([D, S], F32, name="ps_k", tag="sm")
                    for fi in range(2):
                        nc.tensor.matmul(
                            psk[...], wkt_sb[:, h, fi],
                            pztf_sb[:, fi, nb * S:(nb + 1) * S],
                            start=(fi == 0), stop=(fi == 1),
                        )
                    nc.scalar.activation(
                        kt_sb[:, h, nb * S:(nb + 1) * S], psk[...], AF.Identity,
                        bias=bk_sb[:, h],
                    )
                psq = psB.tile([D, S], F32, name="ps_q", tag="sm")
                for fi in range(2):
                    nc.tensor.matmul(
                        psq[...], wqt_sb[:, h, fi], z2t_sb[:, 0, fi],
                        start=(fi == 0), stop=(fi == 1),
                    )
                nc.scalar.activation(qt_sb[:, h], psq[...], AF.Identity, bias=bq_sb[:, h])

            # ---- attention: S^T -> exp -> O^T (denominator folded via V ones-col) ----
            otall_sb = acts.tile([128, 2, S], BF16, name="otall_sb")
            for h in range(NH):
                prow = (h % 2) * 64
                pcol = h // 2
                vaug = []
                for kt in range(KT):
                    psv = psB.tile([128, D + 1], F32, name="ps_v", tag="sm")
                    # ones-col + bias row first so every element's has_written is set
                    nc.tensor.matmul(
                        psv[...], onesr_sb[...], bv_sb[:, h],
                        start=True, stop=False, skip_group_check=True,
                    )
                    for fi in range(2):
                        nc.tensor.matmul(
                            psv[:, 0:D],
                            pztf_sb[:, fi, kt * 128:(kt + 1) * 128],
                            wvt_sb[:, h, fi],
                            start=False, stop=(fi == 1), skip_group_check=True,
                        )
                    va = work.tile([128, D + 1], BF16, name="vaug_sb", tag="vaug", bufs=4)
                    nc.vector.tensor_copy(va[...], psv[...])
                    vaug.append(va)
                pso = psS.tile([D + 1, S], F32, name="ps_ot", tag="ot")
                for kt in range(KT):
                    pss = psA.tile([128, S], F32, name="ps_s", tag="big")
                    nc.tensor.matmul(
                        pss[...],
                        kt_sb[:, h, kt * 128:(kt + 1) * 128],
                        qt_sb[:, h],
                        start=True, stop=True,
                    )
                    ex = work.tile([128, S], BF16, name="exp_sb", tag="exp", bufs=3)
                    nc.scalar.activation(ex[...], pss[...], AF.Exp, scale=0.125)
                    nc.tensor.matmul(
                        pso[...], vaug[kt][...], ex[...],
                        start=(kt == 0), stop=(kt == KT - 1),
                    )
                # normalize rows 0..63 by row 64 (ones-matmul broadcast of 1/denom)
                recip_sb = work.tile([1, S], F32, name="recip_sb", tag="recip", bufs=1)
                nc.vector.reciprocal(recip_sb[...], pso[D:D + 1, :])
                psb = psB.tile([64, S], F32, name="ps_bc", tag="sm")
                nc.tensor.matmul(
                    psb[...], onesr32_sb[...], recip_sb[...], start=True, stop=True
                )
                bc_sb = work.tile([64, S], F32, name="bc_sb", tag="bc", bufs=1)
                nc.vector.tensor_copy(bc_sb[...], psb[...])
                nc.vector.tensor_mul(
                    otall_sb[prow:prow + 64, pcol, :], pso[0:D, :], bc_sb[...]
                )

            # ---- attn_out^T, column sums, AllReduce ----
            ar_in = dram.tile([F, 1], F32, name="ar_in")
            ar_out = dram.tile([F, 1], F32, name="ar_out", addr_space="Shared")
            for fo in range(2):
                psa = psA.tile([128, S], F32, name="ps_ao", tag="big")
                for fi in range(2):
                    nc.tensor.matmul(
                        psa[...], owt_sb[:, fi, fo * 128:(fo + 1) * 128],
                        otall_sb[:, fi, :],
                        start=(fi == 0), stop=(fi == 1),
                    )
                ao_sb = work.tile([128, S], F32, name="ao_sb", tag="ao", bufs=2)
                nc.scalar.activation(ao_sb[...], psa[...], AF.Identity, bias=ob_sb[:, fo])
                aocs_sb = work.tile([128, 1], F32, name="aocs_sb", tag="aocs", bufs=2)
                nc.vector.reduce_sum(aocs_sb[...], ao_sb[...], axis=mybir.AxisListType.X)
                nc.sync.dma_start(ar_in[fo * 128:(fo + 1) * 128, :], aocs_sb[...])
            nc.gpsimd.collective_compute(
                "AllReduce", mybir.AluOpType.add,
                replica_groups=RG, ins=[ar_in.opt()], outs=[ar_out.opt()],
            )
            arcs_sb = work.tile([128, 2, 1], F32, name="arcs_sb", tag="arcs")
            for fo in range(2):
                nc.sync.dma_start(arcs_sb[:, fo], ar_out[fo * 128:(fo + 1) * 128, :])

            # ---- g = (colsum/N) @ sum_w.T + sum_b  (tiny fp32 matmul) ----
            psg = psOne.tile([1, 1], F32, name="ps_g", tag="one")
            for fi in range(2):
                nc.tensor.matmul(
                    psg[...], swt_sb[:, fi], arcs_sb[:, fi],
                    start=(fi == 0), stop=(fi == 1),
                )
            g_sb = work.tile([1, 1], F32, name="g_sb", tag="g")
            nc.scalar.activation(g_sb[...], psg[...], AF.Identity, scale=1.0 / N, bias=sb_sb[...])

            # ---- partial loss ----
            t_sb = work.tile([1, 2, S], F32, name="t_sb", tag="t", bufs=1)
            sp_sb = work.tile([1, 2, S], F32, name="sp_sb", tag="sp", bufs=1)
            red_sb = work.tile([1, 2, 1], F32, name="red_sb", tag="red", bufs=1)
            for v in range(2):
                # softplus(+-t) = ln(1 + exp(+-t));  |t| ~ |g|*|rowsum| stays
                # far from overflow for this model's value ranges
                nc.vector.tensor_scalar_mul(t_sb[:, v], srow_sb[:, v], g_sb[...])
                nc.scalar.activation(sp_sb[:, v], t_sb[:, v], AF.Exp,
                                     scale=(-1.0 if v == 0 else 1.0))
                nc.vector.tensor_scalar_add(sp_sb[:, v], sp_sb[:, v], 1.0)
                nc.scalar.activation(sp_sb[:, v], sp_sb[:, v], AF.Ln)
                nc.vector.reduce_sum(red_sb[:, v], sp_sb[:, v], axis=mybir.AxisListType.X)
            tot_sb = work.tile([1, 1], F32, name="tot_sb", tag="tot")
            nc.vector.tensor_add(tot_sb[...], red_sb[:, 0], red_sb[:, 1])
            nc.sync.dma_start(part_p[...], tot_sb[...])

    _split_excess_waits(nc)
    return nc


def _host_prep(inputs):
    """Build per-core in_maps from full inputs."""
    x = np.asarray(inputs["x"], np.float32)
    ei = np.asarray(inputs["edge_index"]).astype(np.int64)
    W1 = np.asarray(inputs["W1"], np.float32)
    b1 = np.asarray(inputs["b1"], np.float32)
    W2 = np.asarray(inputs["W2"], np.float32)
    b2 = np.asarray(inputs["b2"], np.float32)
    ipw = np.asarray(inputs["in_proj_w"], np.float32)
    ipb = np.asarray(inputs["in_proj_b"], np.float32)
    ow = np.asarray(inputs["out_w"], np.float32)
    ob = np.asarray(inputs["out_b"], np.float32)
    sw = np.asarray(inputs["sum_w"], np.float32)
    sbias = np.asarray(inputs["sum_b"], np.float32)

    bf = ml_dtypes.bfloat16
    row = np.concatenate([ei[0], np.arange(N, dtype=np.int64)])
    col = np.concatenate([ei[1], np.arange(N, dtype=np.int64)])
    deg = np.zeros(N, np.float32)
    np.add.at(deg, col, 1.0)
    dinv = (1.0 / np.sqrt(deg)).astype(np.float32)
    M = np.zeros((N, N), np.float32)
    np.add.at(M, (col, row), dinv[row] * dinv[col])

    x_r = np.ascontiguousarray(x.reshape(2, KT, 128, F)).astype(bf)
    w1_r = np.ascontiguousarray(W1.reshape(2, 128, F)).astype(bf)
    w2_r = np.ascontiguousarray(W2.reshape(2, 128, F)).astype(bf)
    b1_r = np.ascontiguousarray(b1.reshape(2, 128, 1))
    b2_r = np.ascontiguousarray(b2.reshape(2, 128, 1))

    wq, wk, wv = ipw[0:F], ipw[F:2 * F], ipw[2 * F:3 * F]
    bqv, bkv, bvv = ipb[0:F], ipb[F:2 * F], ipb[2 * F:3 * F]
    wqt = np.stack([np.ascontiguousarray(wq[h * D:(h + 1) * D].T).reshape(2, 128, D)
                    for h in range(NH)]).astype(bf)
    wkt = np.stack([np.ascontiguousarray(wk[h * D:(h + 1) * D].T).reshape(2, 128, D)
                    for h in range(NH)]).astype(bf)
    wvt = np.stack([np.ascontiguousarray(wv[h * D:(h + 1) * D].T).reshape(2, 128, D)
                    for h in range(NH)]).astype(bf)
    bq_r = np.stack([bqv[h * D:(h + 1) * D].reshape(D, 1) for h in range(NH)])
    bk_r = np.stack([bkv[h * D:(h + 1) * D].reshape(D, 1) for h in range(NH)])
    bv_r = np.stack([
        np.concatenate([bvv[h * D:(h + 1) * D], [1.0]]).astype(np.float32).reshape(1, D + 1)
        for h in range(NH)
    ]).astype(bf)
    owt = np.ascontiguousarray(ow.T).reshape(2, 128, F).astype(bf)
    ob_r = np.ascontiguousarray(ob.reshape(2, 128, 1))
    swt = np.ascontiguousarray(sw[0].reshape(2, 128, 1))
    sb_r = sbias.reshape(1, 1).astype(np.float32)
    ident = np.eye(128, dtype=bf)
    onesc = np.ones((128, 1), dtype=bf)
    onesr = np.ones((1, 128), dtype=bf)
    onesr32 = np.ones((1, 64), dtype=np.float32)

    common = dict(
        xr=x_r, w1=w1_r, w2=w2_r, b1=b1_r, b2=b2_r,
        wqt=wqt, wkt=wkt, wvt=wvt, bq=bq_r, bk=bk_r, bv=bv_r,
        owt=owt, ob=ob_r, swt=swt, sbias=sb_r,
        ident=ident, onesc=onesc, onesr=onesr, onesr32=onesr32,
    )
    in_maps = []
    for c in range(NC_):
        mt_c = np.ascontiguousarray(
            M[c * S:(c + 1) * S, :].T
        ).reshape(KT, 128, S).astype(bf)
        in_maps.append(dict(common, mt=mt_c))
    return in_maps


_NC_CACHE = None


def get_nc():
    global _NC_CACHE
    if _NC_CACHE is None:
        _NC_CACHE = build_nc()
    return _NC_CACHE


def kernel(**inputs) -> np.ndarray:
    in_maps = _host_prep(inputs)
    res = run_bass_kernel_spmd(get_nc(), in_maps, list(range(NC_)))
    total = sum(float(res.results[c]["partial"][0, 0]) for c in range(NC_))
    return np.float32(total / N)
